# revision 13
# baseline (speedup 1.0000x reference)
"""Trainium2 Bass kernel for DGL-style max-pool aggregation.

Computes: h = feat @ W1.T ; h_N[d] = max over edges e with dst[e]==d of h[src[e]] ;
out = concat([feat, h_N], axis=1).

Strategy (8 NeuronCores, SPMD):
  - dst nodes sharded across cores (12500 each); edges live with their dst
    owner, so segment-max is core-local.
  - Phase 1 (replicated): every core computes the full projection
    h = feat @ W1.T on TensorE, storing four quarter subtables
    htab0..htab3 (25088 rows each, partition-interleaved so stores are 7KB
    contiguous runs).  Quarter granularity keeps rows addressable by the
    int16 indices dma_gather requires, and separate tensors let quarter-q
    gathers start while later quarters are still being projected.
  - Phase 2: per core, edges are bucketed per (quarter, dst), dst nodes
    degree-sorted into groups of 128 with a shared (max-over-cores) padded
    degree so all cores run one instruction stream.  dma_gather fetches
    512B h-rows (8192 indices per instruction — the measured-safe maximum;
    descriptor generation at ~7.3ns/idx is the bottleneck), and a strided
    DVE reduce_max folds each group's padded edge slots.
  - Host work is only index bookkeeping: (dst,src) dedup (max is
    idempotent), sorting/padding, and the final unpermute + concat.
"""

import numpy as np

N_NODES = 100000
D = 128
N_CORES = 8
SH = N_NODES // N_CORES            # 12500 dst nodes per core
G = 128                            # nodes per group
NQ = 4                             # quarter subtables
QROWS = 25088                      # rows per quarter (196*128, < int16 max)
RPQ = QROWS // 128                 # 196 interleave stripes per quarter
NPAD = NQ * QROWS                  # 100352 padded source nodes
CH = 1792                          # phase-1 chunk: 14 subtiles of 128 nodes
CPQ = QROWS // CH                  # 14 chunks per quarter
BATCH_SLOTS = 96                   # 96 slots * 128 = 12288 idxs per gather

_COMPILED = {}
LAST_RESULT = None


def _preprocess(src, dst):
    """Edge bookkeeping -> per-core per-quarter gather schedules."""
    # dedup (dst, src) pairs: max is idempotent, fewer descriptors
    key = dst.astype(np.int64) * N_NODES + src.astype(np.int64)
    key = np.unique(key)
    dst_u = (key // N_NODES).astype(np.int64)
    src_u = (key % N_NODES).astype(np.int64)
    # quarter of each edge's src; edges sorted by (core, quarter, dst, src)
    # == (dst//SH, src//QROWS, dst, src); key2 orders exactly that way
    quar = src_u // QROWS
    key2 = (dst_u // SH) * (NQ * np.int64(SH) * QROWS * 8) \
        + quar * (np.int64(SH) * QROWS * 8) \
        + (dst_u % SH) * (QROWS + 1) + (src_u % QROWS)
    order = np.argsort(key2, kind="stable")
    dst_s, src_s, quar_s = dst_u[order], src_u[order], quar[order]
    # interleaved local row within quarter subtable
    ll = src_s % QROWS
    lrow = ((ll % 128) * RPQ + ll // 128).astype(np.int16)

    # per (core, quarter): node lists, degrees, CSR starts into the sorted list
    ck = dst_s // SH
    # count per (core, quarter, node): use bincount over compound key
    cnt_key = (ck * NQ + quar_s) * SH + (dst_s % SH)
    cnt = np.bincount(cnt_key, minlength=N_CORES * NQ * SH).reshape(
        N_CORES, NQ, SH)

    # group counts shared across cores (SPMD): per quarter
    nnz = (cnt > 0).sum(axis=2)                      # [cores, quarters]
    ng_q = [int(-(-nnz[:, q].max() // G)) for q in range(NQ)]
    gtot = sum(ng_q)

    # per-core sorted node orders and shared group degrees
    orders = [[None] * NQ for _ in range(N_CORES)]
    dg_q = []
    for q in range(NQ):
        ng = ng_q[q]
        gmax = np.zeros((N_CORES, ng), np.int64)
        for k in range(N_CORES):
            c = cnt[k, q]
            nodes = np.nonzero(c)[0]
            o = nodes[np.argsort(c[nodes], kind="stable")]
            pad = ng * G - len(o)
            o = np.concatenate([np.repeat(o[:1], pad), o])
            orders[k][q] = o                          # core-local node ids
            gmax[k] = c[o].reshape(ng, G).max(axis=1)
        dg_q.append(gmax.max(axis=0))

    # batches: (quarter, g0, g1, slot columns), sum dg <= BATCH_SLOTS
    batches = []
    slot_off_q = []
    soff = 0
    for q in range(NQ):
        offs = np.zeros(ng_q[q] + 1, np.int64)
        np.cumsum(dg_q[q], out=offs[1:])
        slot_off_q.append(offs)
        g0 = 0
        while g0 < ng_q[q]:
            g1 = g0 + 1
            while (g1 < ng_q[q] and offs[g1 + 1] - offs[g0] <= BATCH_SLOTS
                   and g1 - g0 < 48):
                g1 += 1
            batches.append((q, g0, g1, int(offs[g1] - offs[g0])))
            g0 = g1
        soff += int(offs[-1])
    s_tot = sum(int(o[-1]) for o in slot_off_q)

    # gather index arrays, wrapped for dma_gather: idx i at [i%16, i//16],
    # replicated over the 8 Q7 cores (128 partitions)
    gidx = np.zeros((N_CORES, 128, 8 * s_tot), np.int16)
    # CSR starts per (core, quarter, node)
    starts = np.zeros(N_CORES * NQ * SH + 1, np.int64)
    np.cumsum(cnt.reshape(-1), out=starts[1:])
    for k in range(N_CORES):
        col0 = 0
        for q in range(NQ):
            offs = slot_off_q[q]
            o = orders[k][q]
            deg = cnt[k, q][o]
            st = starts[(k * NQ + q) * SH + o]
            for g in range(ng_q[q]):
                d = int(dg_q[q][g])
                sl = slice(g * G, (g + 1) * G)
                ei = st[sl][:, None] + np.minimum(
                    np.arange(d)[None, :], (deg[sl] - 1)[:, None])
                rows = lrow[ei]                      # [128, d]
                # slot (c, p) -> idx position i = (col0+offs[g]+c)*128 + p
                i0 = (col0 + int(offs[g])) * 128
                idxs = rows.T.reshape(-1)            # i-ordered: c-major, p
                ii = i0 + np.arange(128 * d)
                gidx[k, ii % 16, ii // 16] = idxs
            col0 += int(offs[-1])
    gidx[:, 16:, :] = np.tile(gidx[:, :16, :], (1, 7, 1))

    return orders, ng_q, dg_q, slot_off_q, tuple(batches), s_tot, gidx, gtot


def _build(ng_q, dg_q, slot_off_q, batches, s_tot, gtot):
    """Build + compile the SPMD Bass program."""
    import concourse.bacc as bacc
    import concourse.tile as tile
    from concourse import mybir

    f32 = mybir.dt.float32
    i16 = mybir.dt.int16

    nc = bacc.Bacc("TRN2", target_bir_lowering=False, debug=False,
                   num_devices=N_CORES)
    featT = nc.dram_tensor("featT", [128, NPAD], f32, kind="ExternalInput").ap()
    w1t = nc.dram_tensor("w1t", [128, 128], f32, kind="ExternalInput").ap()
    gidx = nc.dram_tensor("gidx", [128, 8 * s_tot], i16,
                          kind="ExternalInput").ap()
    hN = nc.dram_tensor("hN", [128, gtot * 128], f32,
                        kind="ExternalOutput").ap()
    htabs = [nc.dram_tensor(f"htab{q}", [QROWS, 128], f32).ap()
             for q in range(NQ)]

    with tile.TileContext(nc) as tc:
        with (
            tc.tile_pool(name="w", bufs=1) as wpool,
            tc.tile_pool(name="ld", bufs=2) as ldpool,
            tc.tile_pool(name="hs", bufs=2) as hspool,
            tc.tile_pool(name="ps", bufs=8, space="PSUM") as pspool,
            tc.tile_pool(name="gx", bufs=1) as gxpool,
            tc.tile_pool(name="msgs", bufs=2) as mpool,
            tc.tile_pool(name="acc", bufs=2) as apool,
        ):
            w1t_sb = wpool.tile([128, 128], f32)
            nc.sync.dma_start(out=w1t_sb[:], in_=w1t)
            gx = gxpool.tile([128, 8 * s_tot], i16)
            nc.sync.dma_start(out=gx[:], in_=gidx)

            # ---- phase 1: h = feat @ W1.T into quarter subtables ----
            for q in range(NQ):
                htv = htabs[q].rearrange("(p r) f -> p r f", p=128)
                for c in range(CPQ):
                    n0 = q * QROWS + c * CH
                    ft = ldpool.tile([128, CH], f32, tag="ft")
                    nc.sync.dma_start(out=ft[:], in_=featT[:, n0:n0 + CH])
                    hstage = hspool.tile([128, CH // 128, 128], f32, tag="hs")
                    for b4 in range(4):             # 4+4+4+2 subtiles
                        nsub = min(4, CH // 128 - b4 * 4)
                        pt = pspool.tile([128, 4, 128], f32, tag="ps")
                        for jj in range(nsub):
                            j = b4 * 4 + jj
                            nc.tensor.matmul(
                                pt[:, jj, :],
                                ft[:, j * 128:(j + 1) * 128],
                                w1t_sb[:],
                                start=True, stop=True,
                            )
                        dstv = hstage[:, b4 * 4:b4 * 4 + nsub, :]
                        if b4 % 2 == 0:
                            nc.vector.tensor_copy(out=dstv, in_=pt[:, :nsub, :])
                        else:
                            nc.scalar.copy(out=dstv, in_=pt[:, :nsub, :])
                    nc.sync.dma_start(
                        out=htv[:, c * (CH // 128):(c + 1) * (CH // 128), :],
                        in_=hstage[:],
                    )

            # ---- phase 2: gather + per-group segment max ----
            gq_base = {}
            acc_base = 0
            for q in range(NQ):
                gq_base[q] = acc_base
                acc_base += ng_q[q]
            colq = np.zeros(NQ + 1, np.int64)
            np.cumsum([int(o[-1]) for o in slot_off_q], out=colq[1:])

            for (q, g0, g1, S) in batches:
                offs = slot_off_q[q]
                col0 = int(colq[q] + offs[g0])
                msgs = mpool.tile([128, BATCH_SLOTS, 128], f32, tag="msgs")
                nc.gpsimd.dma_gather(
                    out_ap=msgs[:, :S, :],
                    in_ap=htabs[q][:, :],
                    idxs_ap=gx[:, 8 * col0:8 * (col0 + S)],
                    num_idxs=128 * S,
                    num_idxs_reg=128 * S,
                    elem_size=128,
                    single_packet=False,
                )
                nb = g1 - g0
                ost = apool.tile([128, 48, 128], f32, tag="ost")
                for gi, g in enumerate(range(g0, g1)):
                    a = int(offs[g] - offs[g0])
                    d = int(dg_q[q][g])
                    view = msgs[:, a:a + d, :].rearrange("p d f -> p f d")
                    nc.vector.tensor_reduce(
                        out=ost[:, gi, :], in_=view,
                        axis=mybir.AxisListType.X, op=mybir.AluOpType.max,
                    )
                gg = gq_base[q] + g0
                nc.sync.dma_start(
                    out=hN[:, gg * 128:(gg + nb) * 128], in_=ost[:, :nb, :]
                )

    nc.compile()
    return nc


def kernel(feat, W1, src, dst):
    feat = np.asarray(feat, np.float32)
    W1 = np.asarray(W1, np.float32)
    src = np.asarray(src, np.int32)
    dst = np.asarray(dst, np.int32)

    (orders, ng_q, dg_q, slot_off_q, batches, s_tot, gidx, gtot) = \
        _preprocess(src, dst)

    key = (s_tot, gtot, tuple(int(x) for q in range(NQ) for x in dg_q[q]),
           batches)
    if key not in _COMPILED:
        _COMPILED[key] = _build(ng_q, dg_q, slot_off_q, batches, s_tot, gtot)
    nc = _COMPILED[key]

    featT = np.zeros((128, NPAD), np.float32)
    featT[:, :N_NODES] = feat.T
    w1t = np.ascontiguousarray(W1.T)

    in_maps = [
        {"featT": featT, "w1t": w1t, "gidx": np.ascontiguousarray(gidx[k])}
        for k in range(N_CORES)
    ]

    from concourse.bass_utils import run_bass_kernel_spmd
    res = run_bass_kernel_spmd(nc, in_maps, list(range(N_CORES)))
    global LAST_RESULT
    LAST_RESULT = res

    out = np.empty((N_NODES, 2 * D), np.float32)
    out[:, :D] = feat
    gq_base = np.zeros(NQ + 1, np.int64)
    np.cumsum(ng_q, out=gq_base[1:])
    for k in range(N_CORES):
        dev = res.results[k]["hN"].reshape(128, gtot, 128)
        acc = np.full((SH, D), -np.inf, np.float32)
        for q in range(NQ):
            ng = ng_q[q]
            block = dev[:, gq_base[q]:gq_base[q] + ng, :]
            rows = block.transpose(1, 0, 2).reshape(ng * G, D)
            tmp = np.full((SH, D), -np.inf, np.float32)
            tmp[orders[k][q]] = rows
            np.maximum(acc, tmp, out=acc)
        out[k * SH:(k + 1) * SH, D:] = acc
    return out


# revision 17
# speedup vs baseline: 1.3025x; 1.3025x over previous
"""Trainium2 Bass kernel for DGL-style max-pool aggregation.

Computes: h = feat @ W1.T ; h_N[d] = max over edges e with dst[e]==d of h[src[e]] ;
out = concat([feat, h_N], axis=1).

Strategy (8 NeuronCores, SPMD):
  - dst nodes sharded across cores (12500 each); edges live with their dst
    owner, so segment-max is core-local.
  - Phase 1 (replicated): every core computes the full projection
    h = feat @ W1.T on TensorE, storing four quarter subtables
    htab0..htab3 (25088 rows each, partition-interleaved so stores are 7KB
    contiguous runs).  Quarter granularity keeps rows addressable by the
    int16 indices dma_gather requires, and separate tensors let quarter-q
    gathers start while later quarters are still being projected.
  - Phase 2: per core, edges are bucketed per (quarter, dst), dst nodes
    degree-sorted into groups of 128 with a shared (max-over-cores) padded
    degree so all cores run one instruction stream.  dma_gather fetches
    512B h-rows (8192 indices per instruction — the measured-safe maximum;
    descriptor generation at ~7.3ns/idx is the bottleneck), and a strided
    DVE reduce_max folds each group's padded edge slots.
  - Host work is only index bookkeeping: (dst,src) dedup (max is
    idempotent), sorting/padding, and the final unpermute + concat.
"""

import numpy as np

N_NODES = 100000
D = 128
N_CORES = 8
SH = N_NODES // N_CORES            # 12500 dst nodes per core
G = 128                            # nodes per group
NQ = 4                             # quarter subtables
QROWS = 25088                      # rows per quarter (196*128, < int16 max)
RPQ = QROWS // 128                 # 196 interleave stripes per quarter
NPAD = NQ * QROWS                  # 100352 padded source nodes
CH = 1792                          # phase-1 chunk: 14 subtiles of 128 nodes
CPQ = QROWS // CH                  # 14 chunks per quarter
BATCH_SLOTS = 64                   # 64 slots * 128 = 8192 idxs per gather

_COMPILED = {}
LAST_RESULT = None


def _preprocess(src, dst):
    """Edge bookkeeping -> per-core per-quarter gather schedules."""
    # dedup (dst, src) pairs: max is idempotent, fewer descriptors
    key = dst.astype(np.int64) * N_NODES + src.astype(np.int64)
    key = np.unique(key)
    dst_u = (key // N_NODES).astype(np.int64)
    src_u = (key % N_NODES).astype(np.int64)
    # quarter of each edge's src; edges sorted by (core, quarter, dst, src)
    # == (dst//SH, src//QROWS, dst, src); key2 orders exactly that way
    quar = src_u // QROWS
    key2 = (dst_u // SH) * (NQ * np.int64(SH) * QROWS * 8) \
        + quar * (np.int64(SH) * QROWS * 8) \
        + (dst_u % SH) * (QROWS + 1) + (src_u % QROWS)
    order = np.argsort(key2, kind="stable")
    dst_s, src_s, quar_s = dst_u[order], src_u[order], quar[order]
    # interleaved local row within quarter subtable
    ll = src_s % QROWS
    lrow = ((ll % 128) * RPQ + ll // 128).astype(np.int16)

    # per (core, quarter): node lists, degrees, CSR starts into the sorted list
    ck = dst_s // SH
    # count per (core, quarter, node): use bincount over compound key
    cnt_key = (ck * NQ + quar_s) * SH + (dst_s % SH)
    cnt = np.bincount(cnt_key, minlength=N_CORES * NQ * SH).reshape(
        N_CORES, NQ, SH)

    # group counts shared across cores (SPMD): per quarter
    nnz = (cnt > 0).sum(axis=2)                      # [cores, quarters]
    ng_q = [int(-(-nnz[:, q].max() // G)) for q in range(NQ)]
    gtot = sum(ng_q)

    # per-core sorted node orders and shared group degrees
    orders = [[None] * NQ for _ in range(N_CORES)]
    dg_q = []
    for q in range(NQ):
        ng = ng_q[q]
        gmax = np.zeros((N_CORES, ng), np.int64)
        for k in range(N_CORES):
            c = cnt[k, q]
            nodes = np.nonzero(c)[0]
            o = nodes[np.argsort(c[nodes], kind="stable")]
            pad = ng * G - len(o)
            o = np.concatenate([np.repeat(o[:1], pad), o])
            orders[k][q] = o                          # core-local node ids
            gmax[k] = c[o].reshape(ng, G).max(axis=1)
        dg_q.append(gmax.max(axis=0))

    # batches: (quarter, g0, g1, slot columns), sum dg <= BATCH_SLOTS
    batches = []
    slot_off_q = []
    soff = 0
    for q in range(NQ):
        offs = np.zeros(ng_q[q] + 1, np.int64)
        np.cumsum(dg_q[q], out=offs[1:])
        slot_off_q.append(offs)
        g0 = 0
        while g0 < ng_q[q]:
            g1 = g0 + 1
            while (g1 < ng_q[q] and offs[g1 + 1] - offs[g0] <= BATCH_SLOTS
                   and g1 - g0 < 64):
                g1 += 1
            batches.append((q, g0, g1, int(offs[g1] - offs[g0])))
            g0 = g1
        soff += int(offs[-1])
    s_tot = sum(int(o[-1]) for o in slot_off_q)

    # gather index arrays, wrapped for dma_gather: idx i at [i%16, i//16],
    # replicated over the 8 Q7 cores (128 partitions)
    gidx = np.zeros((N_CORES, 128, 8 * s_tot), np.int16)
    # CSR starts per (core, quarter, node)
    starts = np.zeros(N_CORES * NQ * SH + 1, np.int64)
    np.cumsum(cnt.reshape(-1), out=starts[1:])
    for k in range(N_CORES):
        col0 = 0
        for q in range(NQ):
            offs = slot_off_q[q]
            o = orders[k][q]
            deg = cnt[k, q][o]
            st = starts[(k * NQ + q) * SH + o]
            for g in range(ng_q[q]):
                d = int(dg_q[q][g])
                sl = slice(g * G, (g + 1) * G)
                ei = st[sl][:, None] + np.minimum(
                    np.arange(d)[None, :], (deg[sl] - 1)[:, None])
                rows = lrow[ei]                      # [128, d]
                # slot (c, p) -> idx position i = (col0+offs[g]+c)*128 + p
                i0 = (col0 + int(offs[g])) * 128
                idxs = rows.T.reshape(-1)            # i-ordered: c-major, p
                ii = i0 + np.arange(128 * d)
                gidx[k, ii % 16, ii // 16] = idxs
            col0 += int(offs[-1])
    gidx[:, 16:, :] = np.tile(gidx[:, :16, :], (1, 7, 1))

    return orders, ng_q, dg_q, slot_off_q, tuple(batches), s_tot, gidx, gtot


def _build(ng_q, dg_q, slot_off_q, batches, s_tot, gtot):
    """Build + compile the SPMD Bass program."""
    import concourse.bacc as bacc
    import concourse.tile as tile
    from concourse import mybir

    f32 = mybir.dt.float32
    i16 = mybir.dt.int16

    nc = bacc.Bacc("TRN2", target_bir_lowering=False, debug=False,
                   num_devices=N_CORES)
    featT = nc.dram_tensor("featT", [128, NPAD], f32, kind="ExternalInput").ap()
    w1t = nc.dram_tensor("w1t", [128, 128], f32, kind="ExternalInput").ap()
    gidx = nc.dram_tensor("gidx", [128, 8 * s_tot], i16,
                          kind="ExternalInput").ap()
    hN = nc.dram_tensor("hN", [128, gtot * 128], f32,
                        kind="ExternalOutput").ap()
    htabs = [nc.dram_tensor(f"htab{q}", [QROWS, 128], f32).ap()
             for q in range(NQ)]

    with tile.TileContext(nc) as tc:
        with (
            tc.tile_pool(name="w", bufs=1) as wpool,
            tc.tile_pool(name="ld", bufs=3) as ldpool,
            tc.tile_pool(name="hs", bufs=3) as hspool,
            tc.tile_pool(name="ps", bufs=8, space="PSUM") as pspool,
            tc.tile_pool(name="gx", bufs=1) as gxpool,
            tc.tile_pool(name="msgs", bufs=2) as mpool,
            tc.tile_pool(name="acc", bufs=2) as apool,
        ):
            w1t_sb = wpool.tile([128, 128], f32)
            nc.sync.dma_start(out=w1t_sb[:], in_=w1t)
            gx = gxpool.tile([128, 8 * s_tot], i16)
            nc.sync.dma_start(out=gx[:], in_=gidx)

            # ---- phase 1: h = feat @ W1.T into quarter subtables ----
            for q in range(NQ):
                htv = htabs[q].rearrange("(p r) f -> p r f", p=128)
                for c in range(CPQ):
                    n0 = q * QROWS + c * CH
                    ft = ldpool.tile([128, CH], f32, tag="ft")
                    nc.sync.dma_start(out=ft[:], in_=featT[:, n0:n0 + CH])
                    hstage = hspool.tile([128, CH // 128, 128], f32, tag="hs")
                    for b4 in range(4):             # 4+4+4+2 subtiles
                        nsub = min(4, CH // 128 - b4 * 4)
                        pt = pspool.tile([128, 4, 128], f32, tag="ps")
                        for jj in range(nsub):
                            j = b4 * 4 + jj
                            nc.tensor.matmul(
                                pt[:, jj, :],
                                ft[:, j * 128:(j + 1) * 128],
                                w1t_sb[:],
                                start=True, stop=True,
                            )
                        dstv = hstage[:, b4 * 4:b4 * 4 + nsub, :]
                        if b4 % 2 == 0:
                            nc.vector.tensor_copy(out=dstv, in_=pt[:, :nsub, :])
                        else:
                            nc.scalar.copy(out=dstv, in_=pt[:, :nsub, :])
                    nc.sync.dma_start(
                        out=htv[:, c * (CH // 128):(c + 1) * (CH // 128), :],
                        in_=hstage[:],
                    )

            # ---- phase 2: gather + per-group segment max ----
            gq_base = {}
            acc_base = 0
            for q in range(NQ):
                gq_base[q] = acc_base
                acc_base += ng_q[q]
            colq = np.zeros(NQ + 1, np.int64)
            np.cumsum([int(o[-1]) for o in slot_off_q], out=colq[1:])

            for (q, g0, g1, S) in batches:
                offs = slot_off_q[q]
                col0 = int(colq[q] + offs[g0])
                msgs = mpool.tile([128, BATCH_SLOTS, 128], f32, tag="msgs")
                nc.gpsimd.dma_gather(
                    out_ap=msgs[:, :S, :],
                    in_ap=htabs[q][:, :],
                    idxs_ap=gx[:, 8 * col0:8 * (col0 + S)],
                    num_idxs=128 * S,
                    num_idxs_reg=128 * S,
                    elem_size=128,
                    single_packet=False,
                )
                nb = g1 - g0
                ost = apool.tile([128, 64, 128], f32, tag="ost")
                for gi, g in enumerate(range(g0, g1)):
                    a = int(offs[g] - offs[g0])
                    d = int(dg_q[q][g])
                    view = msgs[:, a:a + d, :].rearrange("p d f -> p f d")
                    nc.vector.tensor_reduce(
                        out=ost[:, gi, :], in_=view,
                        axis=mybir.AxisListType.X, op=mybir.AluOpType.max,
                    )
                gg = gq_base[q] + g0
                nc.sync.dma_start(
                    out=hN[:, gg * 128:(gg + nb) * 128], in_=ost[:, :nb, :]
                )

    nc.compile()
    return nc


def kernel(feat, W1, src, dst):
    feat = np.asarray(feat, np.float32)
    W1 = np.asarray(W1, np.float32)
    src = np.asarray(src, np.int32)
    dst = np.asarray(dst, np.int32)

    (orders, ng_q, dg_q, slot_off_q, batches, s_tot, gidx, gtot) = \
        _preprocess(src, dst)

    key = (s_tot, gtot, tuple(int(x) for q in range(NQ) for x in dg_q[q]),
           batches)
    if key not in _COMPILED:
        _COMPILED[key] = _build(ng_q, dg_q, slot_off_q, batches, s_tot, gtot)
    nc = _COMPILED[key]

    featT = np.zeros((128, NPAD), np.float32)
    featT[:, :N_NODES] = feat.T
    w1t = np.ascontiguousarray(W1.T)

    in_maps = [
        {"featT": featT, "w1t": w1t, "gidx": np.ascontiguousarray(gidx[k])}
        for k in range(N_CORES)
    ]

    from concourse.bass_utils import run_bass_kernel_spmd
    res = run_bass_kernel_spmd(nc, in_maps, list(range(N_CORES)))
    global LAST_RESULT
    LAST_RESULT = res

    out = np.empty((N_NODES, 2 * D), np.float32)
    out[:, :D] = feat
    gq_base = np.zeros(NQ + 1, np.int64)
    np.cumsum(ng_q, out=gq_base[1:])
    for k in range(N_CORES):
        dev = res.results[k]["hN"].reshape(128, gtot, 128)
        acc = np.full((SH, D), -np.inf, np.float32)
        for q in range(NQ):
            ng = ng_q[q]
            block = dev[:, gq_base[q]:gq_base[q] + ng, :]
            rows = block.transpose(1, 0, 2).reshape(ng * G, D)
            tmp = np.full((SH, D), -np.inf, np.float32)
            tmp[orders[k][q]] = rows
            np.maximum(acc, tmp, out=acc)
        out[k * SH:(k + 1) * SH, D:] = acc
    return out
